# revision 22
# baseline (speedup 1.0000x reference)
"""Differential attention kernel for 8 trn2 NeuronCores.

Sharding: (batch, head-group) over 8 cores. Core d handles batch b=d//4 and
heads [4*(d%4), 4*(d%4)+4). Per core, bf16 matmul path (same PE rate as
fp32r but half LDWEIGHTS + half DMA):
  - inputs land in few BIG DMAs (the HWDGE ring serializes at ~630ns/DMA,
    so 119 small DMAs was a 75us startup wall): full [128,2048] x rows on
    the scalar ring, wqk rows + one packed wv on the sync ring; kz zero
    halves are gpsimd memsets (no DMA at all),
  - projections: k1,k2 (zero-padded per-head kz tiles) and v for ALL chunks
    first, then q per-chunk just-in-time; q bias-adds run on the ACT engine
    (Identity+bias, same table set as Exp) to fill its chunk-boundary idle,
  - scores TRANSPOSED sT[j, i] (keys on partitions): mask bias is a
    per-partition ACT bias; exp via one [128, 1024] ACT op per (key tile, m)
    covering both heads of a pair -- ACT exp (~294us) is the steady-state
    pacer, everything else is scheduled around keeping it fed,
  - uT[65, 512] psum accumulation over key tiles; row 64 = denominators,
  - combine o = u1/dn1 - lam*u2/dn2 via reciprocal_approx_fast + DRAM
    round-trip partition-broadcast; writes bf16 oc pair tiles [128, 512]
    (two heads stacked) so the out-projection runs K=128,
  - out-projection: [128,1024] psum tiles under the score tags (free at
    chunk boundaries - no wait on the combine's u-bank release), outst
    copies on ACT, stores on the scalar DMA ring,
  - outproj(c-1) + q_round(c+1) after chunk c keep the PE dense across the
    boundary while ACT drains.
Host sums the 4 partial outT per batch (+bo) and transposes.
"""
import numpy as np

B, S, D, H = 2, 2048, 1024, 16
DH = D // H          # 64
SCALE = DH ** -0.5   # 0.125
NCORES = 8
HG = 4               # heads per device
KT = D // 128        # 8 contraction tiles over D
MT = D // 128        # 8 col tiles of the qk projection (q1,q2,k1,k2 cols)
NCH = S // 512       # 4 query chunks
JT = S // 128        # 16 key tiles

_BUILD_CACHE = {}


def _build(lam: float):
    from contextlib import ExitStack
    import concourse.mybir as mybir
    import concourse.tile as tile
    from concourse import bacc

    f32 = mybir.dt.float32
    f32r = mybir.dt.float32r
    bf16 = mybir.dt.bfloat16
    Exp = mybir.ActivationFunctionType.Exp
    Identity = mybir.ActivationFunctionType.Identity
    Copy = mybir.ActivationFunctionType.Copy
    mult = mybir.AluOpType.mult
    add = mybir.AluOpType.add

    nc = bacc.Bacc("TRN2", target_bir_lowering=False, debug=False,
                   num_devices=NCORES)

    xt_d = nc.dram_tensor("xt", [D, S], bf16, kind="ExternalInput").ap()
    wqk_d = nc.dram_tensor("wqk", [D, D], bf16, kind="ExternalInput").ap()
    wv_d = nc.dram_tensor("wv", [D, HG * DH], bf16, kind="ExternalInput").ap()
    wo_d = nc.dram_tensor("wo", [HG * DH, D], bf16,
                          kind="ExternalInput").ap()
    bqk_d = nc.dram_tensor("bqk", [128, MT], f32, kind="ExternalInput").ap()
    maskb_d = nc.dram_tensor("maskb", [128, JT], f32,
                             kind="ExternalInput").ap()
    out_d = nc.dram_tensor("outT", [D, S], f32, kind="ExternalOutput").ap()

    with tile.TileContext(nc) as tc, ExitStack() as ctx:
        consts = ctx.enter_context(tc.tile_pool(name="consts", bufs=1))
        qk_pool = ctx.enter_context(tc.tile_pool(name="qk", bufs=1))
        v_pool = ctx.enter_context(tc.tile_pool(name="vp", bufs=1))
        ps = ctx.enter_context(tc.tile_pool(name="ps", bufs=1, space="PSUM"))
        projw = ctx.enter_context(tc.tile_pool(name="projw", bufs=1))
        projx = ctx.enter_context(tc.tile_pool(name="projx", bufs=1))

        bqk_sb = consts.tile([128, MT], f32)
        maskb_sb = consts.tile([128, JT], f32)
        # Wo rows grouped per head PAIR (K=128 out-projection)
        wo_sb = [consts.tile([128, D], bf16, name=f"wo{p}", tag=f"wo{p}")
                 for p in range(2)]
        ones1 = consts.tile([128, 1], bf16)
        nc.vector.memset(ones1, 1.0)
        # K=1 stationary for the matmul partition-broadcast in the combine
        # (plain fp32: the reciprocal's output cannot be f32r-rounded)
        ones64 = consts.tile([1, 64], f32)
        nc.vector.memset(ones64, 1.0)

        # v in [S, HG, DH+1] layout; column DH holds ones (denominator trick)
        v_sb = v_pool.tile([128, JT, HG, DH + 1], bf16)
        nc.vector.tensor_copy(
            out=v_sb[:, :, :, DH:DH + 1],
            in_=ones1[:, None, None, :].broadcast_to([128, JT, HG, 1]))

        # q pair tiles: q_t[m][p], heads 2p (rows 0:64) and 2p+1 (rows 64:128)
        q_t = [[qk_pool.tile([128, S], bf16, name=f"q{m}p{p}",
                             tag=f"q{m}p{p}") for p in range(2)]
               for m in range(2)]
        # zero-padded k tiles: kz[m][hl] has k rows in parity half, 0 in other
        kz = [[qk_pool.tile([128, S], bf16, name=f"kz{m}h{hl}",
                            tag=f"kz{m}h{hl}") for hl in range(HG)]
              for m in range(2)]

        # ---------------- input DMA ----------------
        # x rows on the scalar HWDGE ring, everything else on the sync ring
        wqk_sb = [projw.tile([128, D], bf16, name=f"wqk{k}", tag=f"wqk{k}")
                  for k in range(KT)]
        xt_sb = [projx.tile([128, S], bf16, name=f"xt{k}", tag=f"xt{k}")
                 for k in range(KT)]
        for k in range(KT):
            nc.sync.dma_start(out=wqk_sb[k],
                              in_=wqk_d[k * 128:(k + 1) * 128, :])
            nc.scalar.dma_start(out=xt_sb[k],
                                in_=xt_d[k * 128:(k + 1) * 128, :])
        wv_sb = projw.tile([128, KT, HG * DH], bf16, name="wv", tag="wv")
        nc.sync.dma_start(out=wv_sb,
                          in_=wv_d.rearrange("(k p) c -> p k c", p=128))
        nc.sync.dma_start(out=bqk_sb, in_=bqk_d)
        nc.sync.dma_start(out=maskb_sb, in_=maskb_d)
        # kz zero-halves via gpsimd (no DMA, Pool engine is idle)
        for m in range(2):
            for hl in range(HG):
                zh = 1 - (hl % 2)          # the half that must be zero
                nc.gpsimd.memset(kz[m][hl][zh * 64:(zh + 1) * 64, :], 0.0)
        for p in range(2):
            nc.sync.dma_start(out=wo_sb[p],
                              in_=wo_d[p * 128:(p + 1) * 128, :])

        # ---------------- projection rounds ----------------
        # wqk col blocks: mt 0..3 = q1p0,q1p1,q2p0,q2p1; 4..7 = k1,k1,k2,k2
        def qk_round(c, mts):
            nsl = slice(c * 512, (c + 1) * 512)
            pps = [ps.tile([128, 512], f32, name="accp", tag="acc",
                           bufs=4) for _ in mts]
            for k in range(KT):
                for i, mt in enumerate(mts):
                    nc.tensor.matmul(
                        pps[i],
                        wqk_sb[k][:, mt * 128:(mt + 1) * 128],
                        xt_sb[k][:, nsl],
                        start=(k == 0), stop=(k == KT - 1))
            for i, mt in enumerate(mts):
                pp = pps[i]
                if mt < 4:
                    # ACT engine: fills its chunk-boundary idle window
                    m, p = mt // 2, mt % 2
                    nc.scalar.activation(q_t[m][p][:, nsl], pp, Identity,
                                         bias=bqk_sb[:, mt:mt + 1])
                else:
                    # split ACT/DVE: these copies gate the PSUM bank
                    # rotation, and either engine alone becomes the pacer
                    m, pr = (mt - 4) // 2, (mt - 4) % 2
                    for eps in range(2):
                        hl = 2 * pr + eps
                        esl = slice(eps * 64, (eps + 1) * 64)
                        if eps == 0:
                            nc.scalar.activation(
                                kz[m][hl][esl, nsl], pp[esl, :], Identity,
                                bias=bqk_sb[esl, mt:mt + 1])
                        else:
                            nc.vector.tensor_scalar_add(
                                kz[m][hl][esl, nsl], pp[esl, :],
                                bqk_sb[esl, mt:mt + 1])

        def v_round(c):
            vps = [ps.tile([128, HG * DH], f32, name="accv", tag="acc",
                           bufs=4) for _ in range(4)]
            for k in range(KT):
                for sl in range(4):
                    nc.tensor.matmul(
                        vps[sl],
                        xt_sb[k][:, c * 512 + sl * 128:c * 512 + sl * 128
                                 + 128],
                        wv_sb[:, k, :],
                        start=(k == 0), stop=(k == KT - 1))
            for sl in range(4):
                st = c * 4 + sl
                # alternate DVE/ACT so neither copy chain gates the PSUM
                # bank rotation for the next round
                if sl % 2 == 0:
                    nc.vector.tensor_copy(
                        out=v_sb[:, st, :, 0:DH],
                        in_=vps[sl].rearrange("p (h d) -> p h d", h=HG))
                else:
                    nc.scalar.activation(
                        v_sb[:, st, :, 0:DH],
                        vps[sl].rearrange("p (h d) -> p h d", h=HG), Copy)

        # k and v for every chunk first (attention needs full-S keys);
        # q is projected per-chunk right before its attention loop
        for c in range(NCH):
            qk_round(c, (4, 5, 6, 7))
            v_round(c)
        qk_round(0, (0, 1, 2, 3))

        # ---------------- attention ----------------
        e_pool = ctx.enter_context(tc.tile_pool(name="ep", bufs=3))
        oc_pool = ctx.enter_context(tc.tile_pool(name="oc", bufs=4))
        small = ctx.enter_context(tc.tile_pool(name="small", bufs=2))
        outst_pool = ctx.enter_context(tc.tile_pool(name="outst", bufs=2))
        scr_pool = ctx.enter_context(tc.tile_pool(name="scr", bufs=2,
                                                  space="DRAM"))

        oc_store = {}

        def outproj(c):
            csl_o = slice(c * 512, (c + 1) * 512)
            oc_p = oc_store.pop(c)
            for half in range(4):          # mt pair (2*half, 2*half+1)
                # score tags: free at chunk boundaries, so no wait on the
                # combine's u-bank release
                op = ps.tile([128, 1024], f32, name="accop",
                             tag=f"s{half % 2}", bufs=1)
                for i in range(2):
                    mt = 2 * half + i
                    for p in range(2):
                        nc.tensor.matmul(
                            op[:, i * 512:(i + 1) * 512],
                            wo_sb[p][:, mt * 128:(mt + 1) * 128],
                            oc_p[p],
                            start=(p == 0), stop=(p == 1))
                outst = outst_pool.tile([128, 1024], f32, name="outst",
                                        tag="outst")
                # alternate ACT/DVE copies and sync/scalar store rings so the
                # tail drains in parallel
                if half % 2 == 0:
                    nc.scalar.activation(outst, op, Copy)
                else:
                    nc.vector.tensor_copy(out=outst, in_=op)
                for i in range(2):
                    mt = 2 * half + i
                    dma_eng = nc.scalar if (half + i) % 2 == 0 else nc.sync
                    dma_eng.dma_start(
                        out=out_d[mt * 128:(mt + 1) * 128, csl_o],
                        in_=outst[:, i * 512:(i + 1) * 512])

        for c in range(NCH):
            csl = slice(c * 512, (c + 1) * 512)
            oc_c = [None, None]
            for p in range(2):
                u_tiles = []
                for name in ("u1a", "u1b", "u2a", "u2b"):
                    u_tiles.append(ps.tile([DH + 1, 512], f32, name=name,
                                           tag="acc", bufs=4))
                for j in range(JT):
                    jsl = slice(j * 128, (j + 1) * 128)
                    # split score tiles (2 banks each) so next iteration's
                    # score matmuls overlap this iteration's exp
                    e_m = []
                    for m in range(2):
                        s_ps = ps.tile([128, 1024], f32, name=f"s{m}",
                                       tag=f"s{m}", bufs=1)
                        for eps in range(2):
                            nc.tensor.matmul(
                                s_ps[:, eps * 512:(eps + 1) * 512],
                                kz[m][2 * p + eps][:, jsl],
                                q_t[m][p][:, csl],
                                start=True, stop=True)
                        e_sb = e_pool.tile([128, 1024], bf16, name=f"e{m}",
                                           tag=f"e{m}")
                        nc.scalar.activation(e_sb, s_ps, Exp,
                                             bias=maskb_sb[:, j:j + 1],
                                             scale=SCALE)
                        e_m.append(e_sb)
                    # u accumulation; eps-outer so consecutive matmuls share
                    # the same stationary v tile
                    for eps in range(2):
                        for mi in range(2):
                            nc.tensor.matmul(
                                u_tiles[2 * mi + eps],
                                v_sb[:, j, 2 * p + eps, :],
                                e_m[mi][:, eps * 512:(eps + 1) * 512],
                                start=(j == 0), stop=(j == JT - 1))
                # combine: o_hl = u1/dn1 - lam*u2/dn2.
                # u rows are copied out of PSUM immediately (frees the acc
                # banks for the next p-loop); the 4 denominator rows are
                # staged to one partition row, reciprocal'd, then
                # partition-broadcast via a DRAM round-trip.
                # combine: o_hl = u1/dn1 - lam*u2/dn2.
                # u rows leave PSUM immediately (frees the acc banks for the
                # next p-loop); the 4 denominator rows are staged to one
                # partition row and reciprocal'd. The [1,2048] reciprocal row
                # is then partition-broadcast to 64 rows: mid-chunk (p==0)
                # via a DRAM round-trip DMA (score banks are busy), at a
                # chunk boundary (p==1) via a K=1 matmul into the free score
                # banks -- no DMA hops on the critical path.
                u_sbs = []
                g64 = small.tile([1, 2048], f32, name="g64", tag="g64",
                                 bufs=1)
                for eps in range(2):
                    u1 = u_tiles[0 + eps]
                    u2 = u_tiles[2 + eps]
                    u1_sb = small.tile([64, 512], f32, name="u1_sb",
                                       tag="u1_sb")
                    u2_sb = small.tile([64, 512], f32, name="u2_sb",
                                       tag="u2_sb")
                    if p == 1:
                        nc.scalar.activation(u1_sb, u1[0:DH, :], Copy)
                        nc.scalar.activation(u2_sb, u2[0:DH, :], Copy)
                    else:
                        nc.vector.tensor_copy(out=u1_sb, in_=u1[0:DH, :])
                        nc.vector.tensor_copy(out=u2_sb, in_=u2[0:DH, :])
                    u_sbs.append((u1_sb, u2_sb))
                    nc.vector.tensor_copy(
                        out=g64[:, (2 * eps) * 512:(2 * eps + 1) * 512],
                        in_=u1[DH:DH + 1, :])
                    nc.vector.tensor_copy(
                        out=g64[:, (2 * eps + 1) * 512:(2 * eps + 2) * 512],
                        in_=u2[DH:DH + 1, :])
                rg = small.tile([1, 2048], f32, name="rg", tag="rg", bufs=1)
                nc.vector.reciprocal_approx_fast(out=rg, in_=g64)

                def combine_tail():
                    oc_t = oc_pool.tile([128, 512], bf16, name="oc_t",
                                        tag="oc")
                    if p == 1:
                        # matmul partition-broadcast into the score banks
                        # (512 moving elements per matmul is the ISA limit)
                        bcs = []
                        for eps in range(2):
                            bc_ps = ps.tile([64, 1024], f32, name="bc_ps",
                                            tag=f"s{eps}", bufs=1)
                            for i in range(2):
                                seg = (2 * eps + i) * 512
                                nc.tensor.matmul(
                                    bc_ps[:, i * 512:(i + 1) * 512], ones64,
                                    rg[:, seg:seg + 512],
                                    start=True, stop=True)
                            bcs.append(bc_ps)
                        bc_of = lambda eps, i: bcs[eps][:, i * 512:
                                                        (i + 1) * 512]
                    else:
                        scr = scr_pool.tile([4, 512], f32, name="scr",
                                            tag="scr")
                        nc.sync.dma_start(out=scr, in_=rg)
                        bc = small.tile([64, 4, 512], f32, name="bc",
                                        tag="bc", bufs=1)
                        nc.sync.dma_start(out=bc,
                                          in_=scr.partition_broadcast(64))
                        bc_of = lambda eps, i: bc[:, 2 * eps + i, :]
                    for eps in range(2):
                        u1_sb, u2_sb = u_sbs[eps]
                        t1 = small.tile([64, 512], f32, name="t1", tag="t1")
                        nc.vector.tensor_tensor(t1, u1_sb, bc_of(eps, 0),
                                                mult)
                        t2 = small.tile([64, 512], f32, name="t2", tag="t2")
                        nc.vector.tensor_tensor(t2, u2_sb, bc_of(eps, 1),
                                                mult)
                        # oc half = t1 - lam*t2 (head eps of the pair)
                        nc.vector.scalar_tensor_tensor(
                            out=oc_t[eps * 64:(eps + 1) * 64, :], in0=t2,
                            scalar=-float(lam), in1=t1, op0=mult, op1=add)
                    oc_c[p] = oc_t

                if p == 0:
                    combine_tail()
                else:
                    pend_tail = combine_tail
            # the PREVIOUS chunk's out-projection fills this chunk's combine
            # latency (and its s-bank allocations precede the broadcast
            # matmuls'); the next chunk's q projection keeps the PE dense
            # across the boundary
            if c > 0:
                outproj(c - 1)
            pend_tail()
            oc_store[c] = oc_c
            if c < NCH - 1:
                qk_round(c + 1, (0, 1, 2, 3))
        outproj(NCH - 1)

    nc.compile()
    return nc


def _get_nc(lam: float):
    key = round(float(lam), 8)
    if key not in _BUILD_CACHE:
        _BUILD_CACHE[key] = _build(float(lam))
    return _BUILD_CACHE[key]


def _prep_in_maps(hidden_states, attention_mask, Wq, bq, Wk, bk, Wv, bv, Wo,
                  lam_f):
    import ml_dtypes
    bf = ml_dtypes.bfloat16
    in_maps = []
    for d in range(NCORES):
        b, g = d // 4, d % 4
        gc = slice(g * HG * DH, (g + 1) * HG * DH)   # 256 head-group columns
        xt = np.ascontiguousarray(hidden_states[b].T).astype(bf)
        wqk = np.ascontiguousarray(
            np.concatenate([Wq[:, :D][:, gc], Wq[:, D:][:, gc],
                            Wk[:, :D][:, gc], Wk[:, D:][:, gc]],
                           axis=1)).astype(bf)
        wv = np.ascontiguousarray(Wv[:, gc]).astype(bf)
        wo = np.ascontiguousarray(Wo[gc, :]).astype(bf)
        bqk = np.ascontiguousarray(
            np.concatenate([bq[:D][gc], bq[D:][gc], bk[:D][gc], bk[D:][gc]])
            .reshape(MT, 128).T)
        maskb = np.ascontiguousarray(
            ((1.0 - attention_mask[b]) * -10000.0).reshape(JT, 128).T)
        in_maps.append({"xt": xt, "wqk": wqk, "wv": wv, "wo": wo,
                        "bqk": bqk, "maskb": maskb})
    return in_maps


def kernel(hidden_states, attention_mask, Wq, bq, Wk, bk, Wv, bv, Wo, bo,
           lam):
    hidden_states = np.asarray(hidden_states, dtype=np.float32)
    attention_mask = np.asarray(attention_mask, dtype=np.float32)
    Wq = np.asarray(Wq, dtype=np.float32)
    bq = np.asarray(bq, dtype=np.float32)
    Wk = np.asarray(Wk, dtype=np.float32)
    bk = np.asarray(bk, dtype=np.float32)
    Wv = np.asarray(Wv, dtype=np.float32)
    bv = np.asarray(bv, dtype=np.float32)
    Wo = np.asarray(Wo, dtype=np.float32)
    bo = np.asarray(bo, dtype=np.float32)
    lam_f = float(np.asarray(lam))

    from concourse.bass_utils import run_bass_kernel_spmd

    nc = _get_nc(lam_f)
    in_maps = _prep_in_maps(hidden_states, attention_mask, Wq, bq, Wk, bk,
                            Wv, bv, Wo, lam_f)
    res = run_bass_kernel_spmd(nc, in_maps, core_ids=list(range(NCORES)))

    out = np.zeros((B, S, D), np.float32)
    for d in range(NCORES):
        out[d // 4] += res.results[d]["outT"].astype(np.float32).T
    out += bo
    # v-bias correction is linear: o += (1-lam)*bv @ Wo (exact; bv is zero in
    # the reference setup, so this is a no-op there)
    if np.any(bv != 0.0):
        out += ((1.0 - lam_f) * bv) @ Wo
    return out


# revision 26
# speedup vs baseline: 1.0289x; 1.0289x over previous
"""Differential attention kernel for 8 trn2 NeuronCores.

Sharding: (batch, head-group) over 8 cores. Core d handles batch b=d//4 and
heads [4*(d%4), 4*(d%4)+4). Per core, bf16 matmul path (same PE rate as
fp32r but half LDWEIGHTS + half DMA):
  - inputs land in few BIG DMAs (the HWDGE ring serializes at ~630ns/DMA,
    so 119 small DMAs was a 75us startup wall): full [128,2048] x rows on
    the scalar ring, wqk rows + one packed wv on the sync ring; kz zero
    halves are gpsimd memsets (no DMA at all),
  - projections: k1,k2 (zero-padded per-head kz tiles) and v for ALL chunks
    first, then q per-chunk just-in-time; q bias-adds run on the ACT engine
    (Identity+bias, same table set as Exp) to fill its chunk-boundary idle,
  - scores TRANSPOSED sT[j, i] (keys on partitions): mask bias is a
    per-partition ACT bias; exp via one [128, 1024] ACT op per (key tile, m)
    covering both heads of a pair -- ACT exp (~294us) is the steady-state
    pacer, everything else is scheduled around keeping it fed,
  - uT[65, 512] psum accumulation over key tiles; row 64 = denominators,
  - combine o = u1/dn1 - lam*u2/dn2 via reciprocal_approx_fast + DRAM
    round-trip partition-broadcast; writes bf16 oc pair tiles [128, 512]
    (two heads stacked) so the out-projection runs K=128,
  - out-projection: [128,1024] psum tiles under the score tags (free at
    chunk boundaries - no wait on the combine's u-bank release), outst
    copies on ACT, stores on the scalar DMA ring,
  - outproj(c-1) + q_round(c+1) after chunk c keep the PE dense across the
    boundary while ACT drains.
Host sums the 4 partial outT per batch (+bo) and transposes.
"""
import numpy as np

B, S, D, H = 2, 2048, 1024, 16
DH = D // H          # 64
SCALE = DH ** -0.5   # 0.125
NCORES = 8
HG = 4               # heads per device
KT = D // 128        # 8 contraction tiles over D
MT = D // 128        # 8 col tiles of the qk projection (q1,q2,k1,k2 cols)
NCH = S // 512       # 4 query chunks
JT = S // 128        # 16 key tiles

_BUILD_CACHE = {}


def _build(lam: float):
    from contextlib import ExitStack
    import concourse.mybir as mybir
    import concourse.tile as tile
    from concourse import bacc

    f32 = mybir.dt.float32
    f32r = mybir.dt.float32r
    bf16 = mybir.dt.bfloat16
    Exp = mybir.ActivationFunctionType.Exp
    Identity = mybir.ActivationFunctionType.Identity
    Copy = mybir.ActivationFunctionType.Copy
    mult = mybir.AluOpType.mult
    add = mybir.AluOpType.add

    nc = bacc.Bacc("TRN2", target_bir_lowering=False, debug=False,
                   num_devices=NCORES)

    xt_d = nc.dram_tensor("xt", [D, S], bf16, kind="ExternalInput").ap()
    wqk_d = nc.dram_tensor("wqk", [D, D], bf16, kind="ExternalInput").ap()
    wv_d = nc.dram_tensor("wv", [D, HG * DH], bf16, kind="ExternalInput").ap()
    wo_d = nc.dram_tensor("wo", [HG * DH, D], bf16,
                          kind="ExternalInput").ap()
    bqk_d = nc.dram_tensor("bqk", [128, MT], f32, kind="ExternalInput").ap()
    maskb_d = nc.dram_tensor("maskb", [128, JT], f32,
                             kind="ExternalInput").ap()
    out_d = nc.dram_tensor("outT", [D, S], f32, kind="ExternalOutput").ap()

    with tile.TileContext(nc) as tc, ExitStack() as ctx:
        consts = ctx.enter_context(tc.tile_pool(name="consts", bufs=1))
        qk_pool = ctx.enter_context(tc.tile_pool(name="qk", bufs=1))
        v_pool = ctx.enter_context(tc.tile_pool(name="vp", bufs=1))
        ps = ctx.enter_context(tc.tile_pool(name="ps", bufs=1, space="PSUM"))
        projw = ctx.enter_context(tc.tile_pool(name="projw", bufs=1))
        projx = ctx.enter_context(tc.tile_pool(name="projx", bufs=1))

        bqk_sb = consts.tile([128, MT], f32)
        maskb_sb = consts.tile([128, JT], f32)
        # Wo rows grouped per head PAIR (K=128 out-projection)
        wo_sb = [consts.tile([128, D], bf16, name=f"wo{p}", tag=f"wo{p}")
                 for p in range(2)]
        ones1 = consts.tile([128, 1], bf16)
        nc.vector.memset(ones1, 1.0)
        # K=1 stationary for the matmul partition-broadcast in the combine
        # (plain fp32: the reciprocal's output cannot be f32r-rounded)
        ones64 = consts.tile([1, 64], f32)
        nc.vector.memset(ones64, 1.0)

        # v in [S, HG, DH+1] layout; column DH holds ones (denominator trick)
        v_sb = v_pool.tile([128, JT, HG, DH + 1], bf16)
        nc.vector.tensor_copy(
            out=v_sb[:, :, :, DH:DH + 1],
            in_=ones1[:, None, None, :].broadcast_to([128, JT, HG, 1]))

        # q pair tiles: q_t[m][p], heads 2p (rows 0:64) and 2p+1 (rows 64:128)
        q_t = [[qk_pool.tile([128, S], bf16, name=f"q{m}p{p}",
                             tag=f"q{m}p{p}") for p in range(2)]
               for m in range(2)]
        # zero-padded k tiles: kz[m][hl] has k rows in parity half, 0 in other
        kz = [[qk_pool.tile([128, S], bf16, name=f"kz{m}h{hl}",
                            tag=f"kz{m}h{hl}") for hl in range(HG)]
              for m in range(2)]

        # ---------------- input DMA ----------------
        # x rows on the scalar HWDGE ring, everything else on the sync ring
        wqk_sb = [projw.tile([128, D], bf16, name=f"wqk{k}", tag=f"wqk{k}")
                  for k in range(KT)]
        xt_sb = [projx.tile([128, S], bf16, name=f"xt{k}", tag=f"xt{k}")
                 for k in range(KT)]
        wv_sb = projw.tile([128, KT, HG * DH], bf16, name="wv", tag="wv")
        for k in range(KT):
            nc.sync.dma_start(out=wqk_sb[k],
                              in_=wqk_d[k * 128:(k + 1) * 128, :])
            nc.scalar.dma_start(out=xt_sb[k],
                                in_=xt_d[k * 128:(k + 1) * 128, :])
            if k == 1:
                # wv + biases land before v_round(0) needs them, not behind
                # the whole wqk stream
                nc.sync.dma_start(
                    out=wv_sb, in_=wv_d.rearrange("(k p) c -> p k c", p=128))
                nc.sync.dma_start(out=bqk_sb, in_=bqk_d)
        nc.sync.dma_start(out=maskb_sb, in_=maskb_d)
        # kz zero-halves via gpsimd (no DMA, Pool engine is idle)
        for m in range(2):
            for hl in range(HG):
                zh = 1 - (hl % 2)          # the half that must be zero
                nc.gpsimd.memset(kz[m][hl][zh * 64:(zh + 1) * 64, :], 0.0)
        for p in range(2):
            nc.sync.dma_start(out=wo_sb[p],
                              in_=wo_d[p * 128:(p + 1) * 128, :])

        # ---------------- projection rounds ----------------
        # wqk col blocks: mt 0..3 = q1p0,q1p1,q2p0,q2p1; 4..7 = k1,k1,k2,k2
        def qk_round(c, mts):
            nsl = slice(c * 512, (c + 1) * 512)
            pps = [ps.tile([128, 512], f32, name="accp", tag="acc",
                           bufs=4) for _ in mts]
            for k in range(KT):
                for i, mt in enumerate(mts):
                    nc.tensor.matmul(
                        pps[i],
                        wqk_sb[k][:, mt * 128:(mt + 1) * 128],
                        xt_sb[k][:, nsl],
                        start=(k == 0), stop=(k == KT - 1))
            for i, mt in enumerate(mts):
                pp = pps[i]
                if mt < 4:
                    # ACT engine: fills its chunk-boundary idle window
                    m, p = mt // 2, mt % 2
                    nc.scalar.activation(q_t[m][p][:, nsl], pp, Identity,
                                         bias=bqk_sb[:, mt:mt + 1])
                else:
                    # split ACT/DVE: these copies gate the PSUM bank
                    # rotation, and either engine alone becomes the pacer
                    m, pr = (mt - 4) // 2, (mt - 4) % 2
                    for eps in range(2):
                        hl = 2 * pr + eps
                        esl = slice(eps * 64, (eps + 1) * 64)
                        if eps == 0:
                            nc.scalar.activation(
                                kz[m][hl][esl, nsl], pp[esl, :], Identity,
                                bias=bqk_sb[esl, mt:mt + 1])
                        else:
                            nc.vector.tensor_scalar_add(
                                kz[m][hl][esl, nsl], pp[esl, :],
                                bqk_sb[esl, mt:mt + 1])

        def v_round(c):
            vps = [ps.tile([128, HG * DH], f32, name="accv", tag="acc",
                           bufs=4) for _ in range(4)]
            for k in range(KT):
                for sl in range(4):
                    nc.tensor.matmul(
                        vps[sl],
                        xt_sb[k][:, c * 512 + sl * 128:c * 512 + sl * 128
                                 + 128],
                        wv_sb[:, k, :],
                        start=(k == 0), stop=(k == KT - 1))
            for sl in range(4):
                st = c * 4 + sl
                # alternate DVE/ACT so neither copy chain gates the PSUM
                # bank rotation for the next round
                if sl % 2 == 0:
                    nc.vector.tensor_copy(
                        out=v_sb[:, st, :, 0:DH],
                        in_=vps[sl].rearrange("p (h d) -> p h d", h=HG))
                else:
                    nc.scalar.activation(
                        v_sb[:, st, :, 0:DH],
                        vps[sl].rearrange("p (h d) -> p h d", h=HG), Copy)

        # k and v for every chunk first (attention needs full-S keys);
        # q is projected per-chunk right before its attention loop
        for c in range(NCH):
            qk_round(c, (4, 5, 6, 7))
            v_round(c)
        qk_round(0, (0, 1, 2, 3))

        # ---------------- attention ----------------
        e_pool = ctx.enter_context(tc.tile_pool(name="ep", bufs=3))
        oc_pool = ctx.enter_context(tc.tile_pool(name="oc", bufs=4))
        small = ctx.enter_context(tc.tile_pool(name="small", bufs=2))
        outst_pool = ctx.enter_context(tc.tile_pool(name="outst", bufs=2))
        scr_pool = ctx.enter_context(tc.tile_pool(name="scr", bufs=2,
                                                  space="DRAM"))

        oc_store = {}

        def outproj(c):
            csl_o = slice(c * 512, (c + 1) * 512)
            oc_p = oc_store.pop(c)
            for half in range(4):          # mt pair (2*half, 2*half+1)
                # score tags: free at chunk boundaries, so no wait on the
                # combine's u-bank release
                op = ps.tile([128, 1024], f32, name="accop",
                             tag=f"s{half % 2}", bufs=1)
                for i in range(2):
                    mt = 2 * half + i
                    for p in range(2):
                        nc.tensor.matmul(
                            op[:, i * 512:(i + 1) * 512],
                            wo_sb[p][:, mt * 128:(mt + 1) * 128],
                            oc_p[p],
                            start=(p == 0), stop=(p == 1))
                outst = outst_pool.tile([128, 1024], f32, name="outst",
                                        tag="outst")
                # alternate ACT/DVE copies and sync/scalar store rings so the
                # tail drains in parallel
                if half % 2 == 0:
                    nc.scalar.activation(outst, op, Copy)
                else:
                    nc.vector.tensor_copy(out=outst, in_=op)
                for i in range(2):
                    mt = 2 * half + i
                    dma_eng = nc.scalar if (half + i) % 2 == 0 else nc.sync
                    dma_eng.dma_start(
                        out=out_d[mt * 128:(mt + 1) * 128, csl_o],
                        in_=outst[:, i * 512:(i + 1) * 512])

        for c in range(NCH):
            csl = slice(c * 512, (c + 1) * 512)
            oc_c = [None, None]
            for p in range(2):
                u_tiles = []
                for name in ("u1a", "u1b", "u2a", "u2b"):
                    u_tiles.append(ps.tile([DH + 1, 512], f32, name=name,
                                           tag="acc", bufs=4))
                for j in range(JT):
                    jsl = slice(j * 128, (j + 1) * 128)
                    # split score tiles (2 banks each) so next iteration's
                    # score matmuls overlap this iteration's exp
                    e_m = []
                    for m in range(2):
                        s_ps = ps.tile([128, 1024], f32, name=f"s{m}",
                                       tag=f"s{m}", bufs=1)
                        for eps in range(2):
                            nc.tensor.matmul(
                                s_ps[:, eps * 512:(eps + 1) * 512],
                                kz[m][2 * p + eps][:, jsl],
                                q_t[m][p][:, csl],
                                start=True, stop=True)
                        e_sb = e_pool.tile([128, 1024], bf16, name=f"e{m}",
                                           tag=f"e{m}")
                        nc.scalar.activation(e_sb, s_ps, Exp,
                                             bias=maskb_sb[:, j:j + 1],
                                             scale=SCALE)
                        e_m.append(e_sb)
                    # u accumulation; eps-outer so consecutive matmuls share
                    # the same stationary v tile
                    for eps in range(2):
                        for mi in range(2):
                            nc.tensor.matmul(
                                u_tiles[2 * mi + eps],
                                v_sb[:, j, 2 * p + eps, :],
                                e_m[mi][:, eps * 512:(eps + 1) * 512],
                                start=(j == 0), stop=(j == JT - 1))
                # combine: o_hl = u1/dn1 - lam*u2/dn2.
                # u rows are copied out of PSUM immediately (frees the acc
                # banks for the next p-loop); the 4 denominator rows are
                # staged to one partition row, reciprocal'd, then
                # partition-broadcast via a DRAM round-trip.
                # combine: o_hl = u1/dn1 - lam*u2/dn2.
                # u rows leave PSUM immediately (frees the acc banks for the
                # next p-loop); the 4 denominator rows are staged to one
                # partition row and reciprocal'd. The [1,2048] reciprocal row
                # is then partition-broadcast to 64 rows: mid-chunk (p==0)
                # via a DRAM round-trip DMA (score banks are busy), at a
                # chunk boundary (p==1) via a K=1 matmul into the free score
                # banks -- no DMA hops on the critical path.
                u_sbs = []
                g64 = small.tile([1, 2048], f32, name="g64", tag="g64",
                                 bufs=1)
                for eps in range(2):
                    u1 = u_tiles[0 + eps]
                    u2 = u_tiles[2 + eps]
                    u1_sb = small.tile([64, 512], f32, name="u1_sb",
                                       tag="u1_sb")
                    u2_sb = small.tile([64, 512], f32, name="u2_sb",
                                       tag="u2_sb")
                    if p == 1:
                        nc.scalar.activation(u1_sb, u1[0:DH, :], Copy)
                        nc.scalar.activation(u2_sb, u2[0:DH, :], Copy)
                    else:
                        nc.vector.tensor_copy(out=u1_sb, in_=u1[0:DH, :])
                        nc.vector.tensor_copy(out=u2_sb, in_=u2[0:DH, :])
                    u_sbs.append((u1_sb, u2_sb))
                    nc.vector.tensor_copy(
                        out=g64[:, (2 * eps) * 512:(2 * eps + 1) * 512],
                        in_=u1[DH:DH + 1, :])
                    nc.vector.tensor_copy(
                        out=g64[:, (2 * eps + 1) * 512:(2 * eps + 2) * 512],
                        in_=u2[DH:DH + 1, :])
                rg = small.tile([1, 2048], f32, name="rg", tag="rg", bufs=1)
                nc.vector.reciprocal_approx_fast(out=rg, in_=g64)

                def combine_tail():
                    oc_t = oc_pool.tile([128, 512], bf16, name="oc_t",
                                        tag="oc")
                    if p == 1 and c == NCH - 1:
                        # matmul partition-broadcast into the score banks
                        # (512 moving elements per matmul is the ISA limit)
                        bcs = []
                        for eps in range(2):
                            bc_ps = ps.tile([64, 1024], f32, name="bc_ps",
                                            tag=f"s{eps}", bufs=1)
                            for i in range(2):
                                seg = (2 * eps + i) * 512
                                nc.tensor.matmul(
                                    bc_ps[:, i * 512:(i + 1) * 512], ones64,
                                    rg[:, seg:seg + 512],
                                    start=True, stop=True)
                            bcs.append(bc_ps)
                        bc_of = lambda eps, i: bcs[eps][:, i * 512:
                                                        (i + 1) * 512]
                    else:
                        scr = scr_pool.tile([4, 512], f32, name="scr",
                                            tag="scr")
                        nc.sync.dma_start(out=scr, in_=rg)
                        bc = small.tile([64, 4, 512], f32, name="bc",
                                        tag="bc", bufs=1)
                        nc.sync.dma_start(out=bc,
                                          in_=scr.partition_broadcast(64))
                        bc_of = lambda eps, i: bc[:, 2 * eps + i, :]
                    for eps in range(2):
                        u1_sb, u2_sb = u_sbs[eps]
                        t1 = small.tile([64, 512], f32, name="t1", tag="t1")
                        nc.vector.tensor_tensor(t1, u1_sb, bc_of(eps, 0),
                                                mult)
                        t2 = small.tile([64, 512], f32, name="t2", tag="t2")
                        nc.vector.tensor_tensor(t2, u2_sb, bc_of(eps, 1),
                                                mult)
                        # oc half = t1 - lam*t2 (head eps of the pair)
                        nc.vector.scalar_tensor_tensor(
                            out=oc_t[eps * 64:(eps + 1) * 64, :], in0=t2,
                            scalar=-float(lam), in1=t1, op0=mult, op1=add)
                    oc_c[p] = oc_t

                # only the LAST combine defers its tail past outproj(c-1):
                # its matmul-broadcast borrows the freed score banks and
                # shortens the end-of-kernel chain. Mid-run combines stay
                # inline -- the DMA chain overlaps following work anyway.
                if p == 1 and c == NCH - 1:
                    pend_tail = combine_tail
                else:
                    combine_tail()
                    pend_tail = None
            # the PREVIOUS chunk's out-projection fills this chunk's combine
            # latency; the next chunk's q projection keeps the PE dense
            # across the boundary
            if c > 0:
                outproj(c - 1)
            if pend_tail is not None:
                pend_tail()
            oc_store[c] = oc_c
            if c < NCH - 1:
                qk_round(c + 1, (0, 1, 2, 3))
        outproj(NCH - 1)

    nc.compile()
    return nc


def _get_nc(lam: float):
    key = round(float(lam), 8)
    if key not in _BUILD_CACHE:
        _BUILD_CACHE[key] = _build(float(lam))
    return _BUILD_CACHE[key]


def _prep_in_maps(hidden_states, attention_mask, Wq, bq, Wk, bk, Wv, bv, Wo,
                  lam_f):
    import ml_dtypes
    bf = ml_dtypes.bfloat16
    in_maps = []
    for d in range(NCORES):
        b, g = d // 4, d % 4
        gc = slice(g * HG * DH, (g + 1) * HG * DH)   # 256 head-group columns
        xt = np.ascontiguousarray(hidden_states[b].T).astype(bf)
        wqk = np.ascontiguousarray(
            np.concatenate([Wq[:, :D][:, gc], Wq[:, D:][:, gc],
                            Wk[:, :D][:, gc], Wk[:, D:][:, gc]],
                           axis=1)).astype(bf)
        wv = np.ascontiguousarray(Wv[:, gc]).astype(bf)
        wo = np.ascontiguousarray(Wo[gc, :]).astype(bf)
        bqk = np.ascontiguousarray(
            np.concatenate([bq[:D][gc], bq[D:][gc], bk[:D][gc], bk[D:][gc]])
            .reshape(MT, 128).T)
        maskb = np.ascontiguousarray(
            ((1.0 - attention_mask[b]) * -10000.0).reshape(JT, 128).T)
        in_maps.append({"xt": xt, "wqk": wqk, "wv": wv, "wo": wo,
                        "bqk": bqk, "maskb": maskb})
    return in_maps


def kernel(hidden_states, attention_mask, Wq, bq, Wk, bk, Wv, bv, Wo, bo,
           lam):
    hidden_states = np.asarray(hidden_states, dtype=np.float32)
    attention_mask = np.asarray(attention_mask, dtype=np.float32)
    Wq = np.asarray(Wq, dtype=np.float32)
    bq = np.asarray(bq, dtype=np.float32)
    Wk = np.asarray(Wk, dtype=np.float32)
    bk = np.asarray(bk, dtype=np.float32)
    Wv = np.asarray(Wv, dtype=np.float32)
    bv = np.asarray(bv, dtype=np.float32)
    Wo = np.asarray(Wo, dtype=np.float32)
    bo = np.asarray(bo, dtype=np.float32)
    lam_f = float(np.asarray(lam))

    from concourse.bass_utils import run_bass_kernel_spmd

    nc = _get_nc(lam_f)
    in_maps = _prep_in_maps(hidden_states, attention_mask, Wq, bq, Wk, bk,
                            Wv, bv, Wo, lam_f)
    res = run_bass_kernel_spmd(nc, in_maps, core_ids=list(range(NCORES)))

    out = np.zeros((B, S, D), np.float32)
    for d in range(NCORES):
        out[d // 4] += res.results[d]["outT"].astype(np.float32).T
    out += bo
    # v-bias correction is linear: o += (1-lam)*bv @ Wo (exact; bv is zero in
    # the reference setup, so this is a no-op there)
    if np.any(bv != 0.0):
        out += ((1.0 - lam_f) * bv) @ Wo
    return out
